# revision 2
# baseline (speedup 1.0000x reference)
"""DePatchEmbed Trainium2 kernel.

Full op: x (32, 16384, 256) f32 -> out (32, 64, 256, 256) f32 with
  out[n, c, 2*ih+pi, 2*jw+pj] = x[n, jw*128+ih, c*4+pi*2+pj]

Sharding: pure data-parallel over the batch dim — 4 examples per core on
8 NeuronCores. Per core the op is a local permutation done in one pass:

  load : x[n] -> L[ih; jw, d]  SBUF, partition = ih (1 KiB contiguous runs)
  DVE  : S[ih; cl, pi, w=2jw+pj] <- L[ih; jw, (c,pi,pj)]  (strided copies,
         data never leaves its partition)
  store: S -> out[n, c-block]  (2 KiB contiguous runs)
"""

import json

import numpy as np

import concourse.bass as bass
import concourse.bass_utils
import concourse.bass2jax
import concourse.mybir as mybir
from concourse import tile
from concourse.bass_utils import run_bass_kernel_spmd

F32 = mybir.dt.float32

# ---------------------------------------------------------------------------
# The bundled walrus accepts at most one sync-wait per instruction
# ("Too many sync wait commands" in CoreV3GenImpl::setupSyncWait), but Tile's
# kernel-tail Drain carries one wait per outstanding DMA-sem lane. Rewrite the
# BIR before compilation: split any instruction with N>1 waits into N-1
# single-wait Drains followed by the original instruction with one wait.
_ORIG_COMPILE_BIR = concourse.bass_utils.compile_bir_kernel


def _split_multiwait_bir(bir_json: bytes) -> bytes:
    bir = json.loads(bir_json)
    changed = False
    for fn in bir.get("functions", []):
        for bb in fn.get("blocks", []):
            insts = bb.get("instructions", [])
            out = []
            for inst in insts:
                si = inst.get("sync_info")
                waits = si.get("on_wait", []) if si else []
                if len(waits) > 1:
                    changed = True
                    for k, w in enumerate(waits[:-1]):
                        out.append(
                            {
                                "debug": inst.get("debug", 0),
                                "engine": inst["engine"],
                                "ins": [],
                                "outs": [],
                                "is_reset_sema": False,
                                "name": f"{inst['name']}-sw{k}",
                                "opcode": "Drain",
                                "sync_info": {"on_update": [], "on_wait": [w]},
                            }
                        )
                    si["on_wait"] = [waits[-1]]
                out.append(inst)
            bb["instructions"] = out
    if not changed:
        return bir_json
    return json.dumps(bir).encode()


def _patched_compile_bir_kernel(bir_json, tmpdir, neff_name="file.neff"):
    return _ORIG_COMPILE_BIR(_split_multiwait_bir(bir_json), tmpdir, neff_name)


if getattr(concourse.bass2jax.compile_bir_kernel, "__name__", "") != (
    "_patched_compile_bir_kernel"
):
    concourse.bass2jax.compile_bir_kernel = _patched_compile_bir_kernel
    concourse.bass_utils.compile_bir_kernel = _patched_compile_bir_kernel

N_CORES = 8
N_FULL = 32     # full batch
NB = N_FULL // N_CORES  # examples per core
HG = 128        # H // P
WG = 128        # W // P
C = 64          # channels
P = 2           # patch size
DIM = C * P * P             # 256 floats per patch row
LFREE = WG * DIM            # floats per partition for one example
CB = 8                      # channels per store block
NCB = C // CB
SFREE = CB * P * 256
NJB = 8                     # load chunks per example
JB = WG // NJB


def _build_kernel(nc: bass.Bass, x: bass.AP, out: bass.AP):
    with tile.TileContext(nc) as tc:
        with (
            tc.tile_pool(name="lpool", bufs=1) as lpool,
            tc.tile_pool(name="spool", bufs=3) as spool,
        ):
            for n in range(NB):
                L = lpool.tile([128, LFREE], F32, tag="L")
                xv = x[n].rearrange("(jw ih) d -> ih jw d", ih=HG)
                lv = L.rearrange("p (jw d) -> p jw d", d=DIM)
                for jb in range(NJB):
                    nc.sync.dma_start(
                        out=lv[:, jb * JB : (jb + 1) * JB, :],
                        in_=xv[:, jb * JB : (jb + 1) * JB, :],
                    )
                lshuf = L.rearrange(
                    "p (jw c pi pj) -> p jw c pi pj", jw=WG, c=C, pi=P, pj=P
                )
                ov = out[n].rearrange("c (ih pi) w -> ih c (pi w)", ih=HG)
                for cb in range(NCB):
                    S = spool.tile([128, SFREE], F32, tag="S")
                    sv = S.rearrange(
                        "p (cl pi jw pj) -> p cl pi jw pj", cl=CB, pi=P, jw=WG, pj=P
                    )
                    for pi in range(P):
                        for pj in range(P):
                            src = lshuf[:, :, cb * CB : (cb + 1) * CB, pi, pj]
                            src = src.transpose([0, 2, 1])  # [p, cl, jw]
                            dst = sv[:, :, pi, :, pj]       # [p, cl, jw]
                            nc.vector.tensor_copy(out=dst, in_=src)
                    nc.scalar.dma_start(
                        out=ov[:, cb * CB : (cb + 1) * CB, :],
                        in_=S.rearrange("p (cl piw) -> p cl piw", piw=P * 256),
                    )


_NC_CACHE = None


def _get_program() -> bass.Bass:
    global _NC_CACHE
    if _NC_CACHE is None:
        nc = bass.Bass("TRN2", num_devices=N_CORES)
        x = nc.dram_tensor("x", [NB, WG * HG, DIM], F32, kind="ExternalInput")
        out = nc.dram_tensor(
            "out", [NB, C, HG * P, WG * P], F32, kind="ExternalOutput"
        )
        _build_kernel(nc, x.ap(), out.ap())
        _NC_CACHE = nc
    return _NC_CACHE


def kernel(x: np.ndarray, H=256, W=256, **_unused) -> np.ndarray:
    x = np.ascontiguousarray(x, dtype=np.float32)
    assert x.shape == (N_FULL, WG * HG, DIM), x.shape
    nc = _get_program()
    shards = np.split(x, N_CORES, axis=0)
    in_maps = [{"x": s} for s in shards]
    res = run_bass_kernel_spmd(nc, in_maps, core_ids=list(range(N_CORES)))
    outs = [np.asarray(r["out"]) for r in res.results]
    return np.concatenate(outs, axis=0)
